# revision 1
# baseline (speedup 1.0000x reference)
"""Dot-product attention (no softmax) on 8 TRN2 NeuronCores.

out[b,h] = (q[b,h] @ k[b,h].T) @ v[b,h]  for q,k,v [B,H,L,D] = [2,16,2048,64] f32.

Strategy: matmul associativity -> out = q @ (k.T @ v). KV = k.T@v is [64,64]
per head, so the problem collapses from O(L^2 D) to O(L D^2) flops and becomes
purely memory bound (48 MiB in / 16 MiB out).

Sharding: the 32 (b,h) attention instances are independent; each of the 8
cores handles 4 consecutive heads of the flattened (b*h) axis. No collectives.

Per-core layout trick: a head's [2048, 64] tensor is viewed as [128, 16, 64]
(partition p holds rows 16p..16p+15, 4 KiB contiguous DRAM per partition, so
every DMA is fully coalesced). The KV reduction over L is order-independent,
and the same interleaved row mapping flows through transpose -> matmul ->
store unchanged.
"""

import sys

if "/opt/trn_rl_repo" not in sys.path:
    sys.path.insert(0, "/opt/trn_rl_repo")

from contextlib import ExitStack

import numpy as np

import concourse.bass as bass
import concourse.tile as tile
from concourse import bacc, mybir
from concourse.bass_utils import run_bass_kernel_spmd
from concourse.masks import make_identity

B, H, L, D = 2, 16, 2048, 64
N_CORES = 8
HPC = (B * H) // N_CORES  # heads per core = 4
P = 128
J = L // P  # 16 row-slots per partition
F32 = mybir.dt.float32


def _body(ctx: ExitStack, tc: tile.TileContext, o_d, q_d, k_d, v_d):
    nc = tc.nc

    const_pool = ctx.enter_context(tc.tile_pool(name="const", bufs=1))
    in_pool = ctx.enter_context(tc.tile_pool(name="in", bufs=2))
    qt_pool = ctx.enter_context(tc.tile_pool(name="qt", bufs=4))
    kv_pool = ctx.enter_context(tc.tile_pool(name="kv", bufs=2))
    out_pool = ctx.enter_context(tc.tile_pool(name="out", bufs=2))
    psum_kv = ctx.enter_context(tc.tile_pool(name="psum_kv", bufs=2, space="PSUM"))
    psum_t = ctx.enter_context(tc.tile_pool(name="psum_t", bufs=3, space="PSUM"))
    psum_o = ctx.enter_context(tc.tile_pool(name="psum_o", bufs=3, space="PSUM"))

    ident = const_pool.tile([P, P], F32)
    make_identity(nc, ident[:])

    for h in range(HPC):
        q_sb = in_pool.tile([P, J, D], F32, tag="q")
        k_sb = in_pool.tile([P, J, D], F32, tag="k")
        v_sb = in_pool.tile([P, J, D], F32, tag="v")
        nc.sync.dma_start(k_sb[:], k_d[h].rearrange("(p j) d -> p j d", p=P))
        nc.sync.dma_start(v_sb[:], v_d[h].rearrange("(p j) d -> p j d", p=P))
        nc.sync.dma_start(q_sb[:], q_d[h].rearrange("(p j) d -> p j d", p=P))

        # KV = k.T @ v: accumulate 16 [128,64]x[128,64] matmuls into one PSUM bank.
        kv_ps = psum_kv.tile([D, D], F32)
        for j in range(J):
            nc.tensor.matmul(
                kv_ps[:], k_sb[:, j], v_sb[:, j], start=(j == 0), stop=(j == J - 1)
            )
        kv_sb = kv_pool.tile([D, D], F32)
        nc.any.tensor_copy(kv_sb[:], kv_ps[:])

        # out = q @ KV, one 128-row slab at a time: PE-transpose the slab
        # (fp32 has no DMA transpose), then matmul against KV.
        out_sb = out_pool.tile([P, J, D], F32, tag="o")
        for j in range(J):
            qt_ps = psum_t.tile([D, P], F32, tag="qt_ps")
            nc.tensor.transpose(qt_ps[:], q_sb[:, j], ident[:])
            qt_sb = qt_pool.tile([D, P], F32, tag="qt")
            nc.any.tensor_copy(qt_sb[:], qt_ps[:])
            o_ps = psum_o.tile([P, D], F32, tag="o_ps")
            nc.tensor.matmul(o_ps[:], qt_sb[:], kv_sb[:], start=True, stop=True)
            nc.any.tensor_copy(out_sb[:, j], o_ps[:])
        nc.sync.dma_start(o_d[h].rearrange("(p j) d -> p j d", p=P), out_sb[:])


def build():
    nc = bacc.Bacc("TRN2", target_bir_lowering=False, debug=False)
    q_d = nc.dram_tensor("q", [HPC, L, D], F32, kind="ExternalInput").ap()
    k_d = nc.dram_tensor("k", [HPC, L, D], F32, kind="ExternalInput").ap()
    v_d = nc.dram_tensor("v", [HPC, L, D], F32, kind="ExternalInput").ap()
    o_d = nc.dram_tensor("out", [HPC, L, D], F32, kind="ExternalOutput").ap()
    with tile.TileContext(nc) as tc, ExitStack() as ctx:
        _body(ctx, tc, o_d, q_d, k_d, v_d)
    nc.compile()
    return nc


_NC = None


def _get_nc():
    global _NC
    if _NC is None:
        _NC = build()
    return _NC


def make_in_maps(q, k, v):
    qf = np.ascontiguousarray(np.asarray(q, dtype=np.float32).reshape(B * H, L, D))
    kf = np.ascontiguousarray(np.asarray(k, dtype=np.float32).reshape(B * H, L, D))
    vf = np.ascontiguousarray(np.asarray(v, dtype=np.float32).reshape(B * H, L, D))
    return [
        {
            "q": np.ascontiguousarray(qf[c * HPC : (c + 1) * HPC]),
            "k": np.ascontiguousarray(kf[c * HPC : (c + 1) * HPC]),
            "v": np.ascontiguousarray(vf[c * HPC : (c + 1) * HPC]),
        }
        for c in range(N_CORES)
    ]


def run_sharded(q, k, v, **spmd_kwargs):
    """Run on all 8 cores; returns (full_output, BassKernelResults)."""
    nc = _get_nc()
    res = run_bass_kernel_spmd(
        nc, make_in_maps(q, k, v), core_ids=list(range(N_CORES)), **spmd_kwargs
    )
    shards = [np.asarray(res.results[c]["out"]) for c in range(N_CORES)]
    out = np.concatenate(shards, axis=0).reshape(B, H, L, D).astype(np.float32)
    return out, res


def kernel(q, k, v):
    out, _ = run_sharded(q, k, v)
    return out


# revision 5
# speedup vs baseline: 1.6286x; 1.6286x over previous
"""Dot-product attention (no softmax) on 8 TRN2 NeuronCores.

out[b,h] = (q[b,h] @ k[b,h].T) @ v[b,h]  for q,k,v [B,H,L,D] = [2,16,2048,64] f32.

Strategy: matmul associativity -> out = q @ (k.T @ v). KV = k.T@v is [64,64]
per head, so the problem collapses from O(L^2 D) to O(L D^2) flops and becomes
purely memory bound (48 MiB in / 16 MiB out).

Sharding: the 32 (b,h) attention instances are independent; each of the 8
cores handles 4 consecutive heads of the flattened (b*h) axis. No collectives.

Per-core layout trick: a head's [2048, 64] tensor is viewed as [128, 16, 64]
(partition p holds rows 16p..16p+15, 4 KiB contiguous DRAM per partition, so
every DMA is fully coalesced). The KV reduction over L is order-independent,
and the same interleaved row mapping flows through transpose -> matmul ->
store unchanged.
"""

import sys

if "/opt/trn_rl_repo" not in sys.path:
    sys.path.insert(0, "/opt/trn_rl_repo")

from contextlib import ExitStack

import numpy as np

import concourse.bass as bass
import concourse.tile as tile
from concourse import bacc, mybir
from concourse.bass_utils import run_bass_kernel_spmd
from concourse.masks import make_identity

B, H, L, D = 2, 16, 2048, 64
N_CORES = 8
HPC = (B * H) // N_CORES  # heads per core = 4
P = 128
J = L // P  # 16 row-slots per partition
F32 = mybir.dt.float32


def _body(ctx: ExitStack, tc: tile.TileContext, o_d, q_d, k_d, v_d):
    nc = tc.nc

    const_pool = ctx.enter_context(tc.tile_pool(name="const", bufs=1))
    in_pool = ctx.enter_context(tc.tile_pool(name="in", bufs=2))
    qt_pool = ctx.enter_context(tc.tile_pool(name="qt", bufs=4))
    kv_pool = ctx.enter_context(tc.tile_pool(name="kv", bufs=2))
    out_pool = ctx.enter_context(tc.tile_pool(name="out", bufs=2))
    psum_kv = ctx.enter_context(tc.tile_pool(name="psum_kv", bufs=1, space="PSUM"))
    psum_s = ctx.enter_context(tc.tile_pool(name="psum_s", bufs=1, space="PSUM"))
    psum_t = ctx.enter_context(tc.tile_pool(name="psum_t", bufs=2, space="PSUM"))
    psum_o = ctx.enter_context(tc.tile_pool(name="psum_o", bufs=2, space="PSUM"))

    ident = const_pool.tile([P, P], F32)
    make_identity(nc, ident[:])

    # ones_dbl[p, m] = 1 iff p == m (mod 64): one matmul against it both sums
    # the two column-tiled KV halves and replicates the result to partitions
    # 64..127 (needed as the row-group-1 operand of the row-tiled out matmuls).
    ones_dbl = const_pool.tile([P, P], F32)
    nc.gpsimd.memset(ones_dbl[:], 0.0)
    for off in (-64, 0, 64):
        nc.gpsimd.affine_select(
            out=ones_dbl[:],
            in_=ones_dbl[:],
            compare_op=mybir.AluOpType.not_equal,
            fill=1.0,
            base=-off,
            pattern=[[-1, P]],
            channel_multiplier=1,
        )

    for h in range(HPC):
        q_sb = in_pool.tile([P, J, D], F32, tag="q")
        k_sb = in_pool.tile([P, J, D], F32, tag="k")
        v_sb = in_pool.tile([P, J, D], F32, tag="v")
        nc.sync.dma_start(k_sb[:], k_d[h].rearrange("(p j) d -> p j d", p=P))
        nc.sync.dma_start(v_sb[:], v_d[h].rearrange("(p j) d -> p j d", p=P))
        nc.sync.dma_start(q_sb[:], q_d[h].rearrange("(p j) d -> p j d", p=P))

        # KV = k.T @ v, column-tiled: even j-slots accumulate into PE columns
        # 0..63 (psum partitions 0..63), odd slots into columns 64..127, so
        # the two matmuls of a pair run concurrently in the array.
        kv_ps = psum_kv.tile([P, D], F32)
        for jp in range(J // 2):
            nc.tensor.matmul(
                kv_ps[0:D],
                k_sb[:, 2 * jp],
                v_sb[:, 2 * jp],
                start=(jp == 0),
                stop=(jp == J // 2 - 1),
                tile_position=(0, 0),
                skip_group_check=True,
            )
            nc.tensor.matmul(
                kv_ps[D : 2 * D],
                k_sb[:, 2 * jp + 1],
                v_sb[:, 2 * jp + 1],
                start=(jp == 0),
                stop=(jp == J // 2 - 1),
                tile_position=(0, D),
                skip_group_check=True,
            )
        kv_raw = kv_pool.tile([P, D], F32, tag="kv_raw")
        nc.any.tensor_copy(kv_raw[:], kv_ps[:])
        kv_st_ps = psum_s.tile([P, D], F32)
        nc.tensor.matmul(kv_st_ps[:], ones_dbl[:], kv_raw[:], start=True, stop=True)
        kv_stack = kv_pool.tile([P, D], F32, tag="kv_stack")
        nc.any.tensor_copy(kv_stack[:], kv_st_ps[:])

        # out = q @ KV. Transpose q two 64-wide slabs at a time (one PE
        # transpose yields qT for slots 2jp and 2jp+1 stacked on partition
        # halves), then two row-tiled matmuls (row groups 0..63 / 64..127)
        # run concurrently, writing into one pair-packed PSUM tile.
        out_sb = out_pool.tile([P, J, D], F32, tag="o")
        for jp in range(J // 2):
            qt_ps = psum_t.tile([P, P], F32, tag="qt_ps")
            nc.tensor.transpose(qt_ps[:], q_sb[:, 2 * jp : 2 * jp + 2], ident[:])
            qt_sb = qt_pool.tile([P, P], F32, tag="qt")
            nc.any.tensor_copy(qt_sb[:], qt_ps[:])
            o_ps_a = psum_o.tile([P, D], F32, tag="o_ps_a")
            o_ps_b = psum_o.tile([P, D], F32, tag="o_ps_b")
            nc.tensor.matmul(
                o_ps_a[:],
                qt_sb[0:D],
                kv_stack[0:D],
                start=True,
                stop=True,
                tile_position=(0, 0),
            )
            nc.tensor.matmul(
                o_ps_b[:],
                qt_sb[D : 2 * D],
                kv_stack[D : 2 * D],
                start=True,
                stop=True,
                tile_position=(D, 0),
            )
            nc.any.tensor_copy(out_sb[:, 2 * jp], o_ps_a[:])
            nc.any.tensor_copy(out_sb[:, 2 * jp + 1], o_ps_b[:])
        nc.sync.dma_start(o_d[h].rearrange("(p j) d -> p j d", p=P), out_sb[:])


def build():
    nc = bacc.Bacc("TRN2", target_bir_lowering=False, debug=False)
    q_d = nc.dram_tensor("q", [HPC, L, D], F32, kind="ExternalInput").ap()
    k_d = nc.dram_tensor("k", [HPC, L, D], F32, kind="ExternalInput").ap()
    v_d = nc.dram_tensor("v", [HPC, L, D], F32, kind="ExternalInput").ap()
    o_d = nc.dram_tensor("out", [HPC, L, D], F32, kind="ExternalOutput").ap()
    with tile.TileContext(nc) as tc, ExitStack() as ctx:
        _body(ctx, tc, o_d, q_d, k_d, v_d)
    nc.compile()
    return nc


_NC = None


def _get_nc():
    global _NC
    if _NC is None:
        _NC = build()
    return _NC


def make_in_maps(q, k, v):
    qf = np.ascontiguousarray(np.asarray(q, dtype=np.float32).reshape(B * H, L, D))
    kf = np.ascontiguousarray(np.asarray(k, dtype=np.float32).reshape(B * H, L, D))
    vf = np.ascontiguousarray(np.asarray(v, dtype=np.float32).reshape(B * H, L, D))
    return [
        {
            "q": np.ascontiguousarray(qf[c * HPC : (c + 1) * HPC]),
            "k": np.ascontiguousarray(kf[c * HPC : (c + 1) * HPC]),
            "v": np.ascontiguousarray(vf[c * HPC : (c + 1) * HPC]),
        }
        for c in range(N_CORES)
    ]


def run_sharded(q, k, v, **spmd_kwargs):
    """Run on all 8 cores; returns (full_output, BassKernelResults)."""
    nc = _get_nc()
    res = run_bass_kernel_spmd(
        nc, make_in_maps(q, k, v), core_ids=list(range(N_CORES)), **spmd_kwargs
    )
    shards = [np.asarray(res.results[c]["out"]) for c in range(N_CORES)]
    out = np.concatenate(shards, axis=0).reshape(B, H, L, D).astype(np.float32)
    return out, res


def kernel(q, k, v):
    out, _ = run_sharded(q, k, v)
    return out


# revision 7
# speedup vs baseline: 1.6554x; 1.0164x over previous
"""Dot-product attention (no softmax) on 8 TRN2 NeuronCores.

out[b,h] = (q[b,h] @ k[b,h].T) @ v[b,h]  for q,k,v [B,H,L,D] = [2,16,2048,64] f32.

Strategy: matmul associativity -> out = q @ (k.T @ v). KV = k.T@v is [64,64]
per head, so the problem collapses from O(L^2 D) to O(L D^2) flops and becomes
purely memory bound (48 MiB in / 16 MiB out).

Sharding: the 32 (b,h) attention instances are independent; each of the 8
cores handles 4 consecutive heads of the flattened (b*h) axis. No collectives.

Per-core layout trick: a head's [2048, 64] tensor is viewed as [128, 16, 64]
(partition p holds rows 16p..16p+15, 4 KiB contiguous DRAM per partition, so
every DMA is fully coalesced). The KV reduction over L is order-independent,
and the same interleaved row mapping flows through transpose -> matmul ->
store unchanged.
"""

import sys

if "/opt/trn_rl_repo" not in sys.path:
    sys.path.insert(0, "/opt/trn_rl_repo")

from contextlib import ExitStack

import numpy as np

import os

import concourse.bass as bass
import concourse.tile as tile
from concourse import bacc, bass_utils, mybir
from concourse.bass_utils import run_bass_kernel_spmd
from concourse.masks import make_identity

if os.environ.get("ATTN_LDW_OPT") == "1" and not hasattr(bass_utils, "_attn_ldw_patch"):
    bass_utils._attn_ldw_patch = bass_utils.run_command

    def _run_command_ldw(cmd, *a, **kw):
        if isinstance(cmd, list):
            cmd = [
                "--enable-ldw-opt=true" if c == "--enable-ldw-opt=false" else c
                for c in cmd
            ]
        return bass_utils._attn_ldw_patch(cmd, *a, **kw)

    bass_utils.run_command = _run_command_ldw

B, H, L, D = 2, 16, 2048, 64
N_CORES = 8
HPC = (B * H) // N_CORES  # heads per core = 4
P = 128
J = L // P  # 16 row-slots per partition
F32 = mybir.dt.float32


def _body(ctx: ExitStack, tc: tile.TileContext, o_d, q_d, k_d, v_d):
    nc = tc.nc

    tag = "_ldw" if os.environ.get("ATTN_LDW_OPT") == "1" else ""
    const_pool = ctx.enter_context(tc.tile_pool(name="const" + tag, bufs=1))
    in_pool = ctx.enter_context(tc.tile_pool(name="in", bufs=2))
    qt_pool = ctx.enter_context(tc.tile_pool(name="qt", bufs=4))
    kv_pool = ctx.enter_context(tc.tile_pool(name="kv", bufs=2))
    out_pool = ctx.enter_context(tc.tile_pool(name="out", bufs=2))
    psum_kv = ctx.enter_context(tc.tile_pool(name="psum_kv", bufs=1, space="PSUM"))
    psum_s = ctx.enter_context(tc.tile_pool(name="psum_s", bufs=1, space="PSUM"))
    psum_t = ctx.enter_context(tc.tile_pool(name="psum_t", bufs=2, space="PSUM"))
    psum_o = ctx.enter_context(tc.tile_pool(name="psum_o", bufs=2, space="PSUM"))

    ident = const_pool.tile([P, P], F32)
    make_identity(nc, ident[:])

    # ones_dbl[p, m] = 1 iff p == m (mod 64): one matmul against it both sums
    # the two column-tiled KV halves and replicates the result to partitions
    # 64..127 (needed as the row-group-1 operand of the row-tiled out matmuls).
    ones_dbl = const_pool.tile([P, P], F32)
    nc.gpsimd.memset(ones_dbl[:], 0.0)
    for off in (-64, 0, 64):
        nc.gpsimd.affine_select(
            out=ones_dbl[:],
            in_=ones_dbl[:],
            compare_op=mybir.AluOpType.not_equal,
            fill=1.0,
            base=-off,
            pattern=[[-1, P]],
            channel_multiplier=1,
        )

    for h in range(HPC):
        q_sb = in_pool.tile([P, J, D], F32, tag="q")
        k_sb = in_pool.tile([P, J, D], F32, tag="k")
        v_sb = in_pool.tile([P, J, D], F32, tag="v")
        nc.sync.dma_start(k_sb[:], k_d[h].rearrange("(p j) d -> p j d", p=P))
        nc.sync.dma_start(v_sb[:], v_d[h].rearrange("(p j) d -> p j d", p=P))
        nc.sync.dma_start(q_sb[:], q_d[h].rearrange("(p j) d -> p j d", p=P))

        # KV = k.T @ v, column-tiled: even j-slots accumulate into PE columns
        # 0..63 (psum partitions 0..63), odd slots into columns 64..127, so
        # the two matmuls of a pair run concurrently in the array.
        kv_ps = psum_kv.tile([P, D], F32)
        for jp in range(J // 2):
            nc.tensor.matmul(
                kv_ps[0:D],
                k_sb[:, 2 * jp],
                v_sb[:, 2 * jp],
                start=(jp == 0),
                stop=(jp == J // 2 - 1),
                tile_position=(0, 0),
                skip_group_check=True,
            )
            nc.tensor.matmul(
                kv_ps[D : 2 * D],
                k_sb[:, 2 * jp + 1],
                v_sb[:, 2 * jp + 1],
                start=(jp == 0),
                stop=(jp == J // 2 - 1),
                tile_position=(0, D),
                skip_group_check=True,
            )
        kv_raw = kv_pool.tile([P, D], F32, tag="kv_raw")
        nc.any.tensor_copy(kv_raw[:], kv_ps[:])
        kv_st_ps = psum_s.tile([P, D], F32)
        nc.tensor.matmul(kv_st_ps[:], ones_dbl[:], kv_raw[:], start=True, stop=True)
        kv_stack = kv_pool.tile([P, D], F32, tag="kv_stack")
        nc.any.tensor_copy(kv_stack[:], kv_st_ps[:])

        # out = q @ KV. Transpose q two 64-wide slabs at a time (one PE
        # transpose yields qT for slots 2jp and 2jp+1 stacked on partition
        # halves), then two row-tiled matmuls (row groups 0..63 / 64..127)
        # run concurrently, writing into one pair-packed PSUM tile.
        out_sb = out_pool.tile([P, J, D], F32, tag="o")
        for jp in range(J // 2):
            qt_ps = psum_t.tile([P, P], F32, tag="qt_ps")
            nc.tensor.transpose(qt_ps[:], q_sb[:, 2 * jp : 2 * jp + 2], ident[:])
            qt_sb = qt_pool.tile([P, P], F32, tag="qt")
            nc.any.tensor_copy(qt_sb[:], qt_ps[:])
            o_ps_a = psum_o.tile([P, D], F32, tag="o_ps_a")
            o_ps_b = psum_o.tile([P, D], F32, tag="o_ps_b")
            nc.tensor.matmul(
                o_ps_a[:],
                qt_sb[0:D],
                kv_stack[0:D],
                start=True,
                stop=True,
                tile_position=(0, 0),
            )
            nc.tensor.matmul(
                o_ps_b[:],
                qt_sb[D : 2 * D],
                kv_stack[D : 2 * D],
                start=True,
                stop=True,
                tile_position=(D, 0),
            )
            nc.any.tensor_copy(out_sb[:, 2 * jp], o_ps_a[:])
            nc.any.tensor_copy(out_sb[:, 2 * jp + 1], o_ps_b[:])
        nc.sync.dma_start(o_d[h].rearrange("(p j) d -> p j d", p=P), out_sb[:])


def build():
    nc = bacc.Bacc("TRN2", target_bir_lowering=False, debug=False)
    q_d = nc.dram_tensor("q", [HPC, L, D], F32, kind="ExternalInput").ap()
    k_d = nc.dram_tensor("k", [HPC, L, D], F32, kind="ExternalInput").ap()
    v_d = nc.dram_tensor("v", [HPC, L, D], F32, kind="ExternalInput").ap()
    o_d = nc.dram_tensor("out", [HPC, L, D], F32, kind="ExternalOutput").ap()
    with tile.TileContext(nc) as tc, ExitStack() as ctx:
        _body(ctx, tc, o_d, q_d, k_d, v_d)
    nc.compile()
    return nc


_NC = None


def _get_nc():
    global _NC
    if _NC is None:
        _NC = build()
    return _NC


def make_in_maps(q, k, v):
    qf = np.ascontiguousarray(np.asarray(q, dtype=np.float32).reshape(B * H, L, D))
    kf = np.ascontiguousarray(np.asarray(k, dtype=np.float32).reshape(B * H, L, D))
    vf = np.ascontiguousarray(np.asarray(v, dtype=np.float32).reshape(B * H, L, D))
    return [
        {
            "q": np.ascontiguousarray(qf[c * HPC : (c + 1) * HPC]),
            "k": np.ascontiguousarray(kf[c * HPC : (c + 1) * HPC]),
            "v": np.ascontiguousarray(vf[c * HPC : (c + 1) * HPC]),
        }
        for c in range(N_CORES)
    ]


def run_sharded(q, k, v, **spmd_kwargs):
    """Run on all 8 cores; returns (full_output, BassKernelResults)."""
    nc = _get_nc()
    res = run_bass_kernel_spmd(
        nc, make_in_maps(q, k, v), core_ids=list(range(N_CORES)), **spmd_kwargs
    )
    shards = [np.asarray(res.results[c]["out"]) for c in range(N_CORES)]
    out = np.concatenate(shards, axis=0).reshape(B, H, L, D).astype(np.float32)
    return out, res


def kernel(q, k, v):
    out, _ = run_sharded(q, k, v)
    return out


# revision 11
# speedup vs baseline: 1.8442x; 1.1141x over previous
"""Dot-product attention (no softmax) on 8 TRN2 NeuronCores.

out[b,h] = (q[b,h] @ k[b,h].T) @ v[b,h]  for q,k,v [B,H,L,D] = [2,16,2048,64] f32.

Strategy: matmul associativity -> out = q @ (k.T @ v). KV = k.T@v is [64,64]
per head, so the problem collapses from O(L^2 D) to O(L D^2) flops and becomes
purely memory bound (48 MiB in / 16 MiB out).

Sharding: the 32 (b,h) attention instances are independent; each of the 8
cores handles 4 consecutive heads of the flattened (b*h) axis. No collectives.

Per-core layout trick: a head's [2048, 64] tensor is viewed as [128, 16, 64]
(partition p holds rows 16p..16p+15, 4 KiB contiguous DRAM per partition, so
every DMA is fully coalesced). The KV reduction over L is order-independent,
and the same interleaved row mapping flows through transpose -> matmul ->
store unchanged.
"""

import sys

if "/opt/trn_rl_repo" not in sys.path:
    sys.path.insert(0, "/opt/trn_rl_repo")

from contextlib import ExitStack

import numpy as np

import os

import concourse.bass as bass
import concourse.tile as tile
from concourse import bacc, bass_utils, mybir
from concourse.bass_utils import run_bass_kernel_spmd
from concourse.masks import make_identity

if os.environ.get("ATTN_LDW_OPT") == "1" and not hasattr(bass_utils, "_attn_ldw_patch"):
    bass_utils._attn_ldw_patch = bass_utils.run_command

    def _run_command_ldw(cmd, *a, **kw):
        if isinstance(cmd, list):
            cmd = [
                "--enable-ldw-opt=true" if c == "--enable-ldw-opt=false" else c
                for c in cmd
            ]
        return bass_utils._attn_ldw_patch(cmd, *a, **kw)

    bass_utils.run_command = _run_command_ldw

B, H, L, D = 2, 16, 2048, 64
N_CORES = 8
HPC = (B * H) // N_CORES  # heads per core = 4
P = 128
J = L // P  # 16 row-slots per partition
F32 = mybir.dt.float32


def _body(ctx: ExitStack, tc: tile.TileContext, o_d, q_d, k_d, v_d):
    nc = tc.nc

    tag = "_ldw" if os.environ.get("ATTN_LDW_OPT") == "1" else ""
    const_pool = ctx.enter_context(tc.tile_pool(name="const" + tag, bufs=1))
    in_pool = ctx.enter_context(tc.tile_pool(name="in", bufs=3))
    qt_pool = ctx.enter_context(tc.tile_pool(name="qt", bufs=10))
    kv_pool = ctx.enter_context(tc.tile_pool(name="kv", bufs=2))
    out_pool = ctx.enter_context(tc.tile_pool(name="out", bufs=2))
    psum_kv = ctx.enter_context(tc.tile_pool(name="psum_kv", bufs=1, space="PSUM"))
    psum_s = ctx.enter_context(tc.tile_pool(name="psum_s", bufs=1, space="PSUM"))
    psum_t = ctx.enter_context(tc.tile_pool(name="psum_t", bufs=2, space="PSUM"))
    psum_o = ctx.enter_context(tc.tile_pool(name="psum_o", bufs=2, space="PSUM"))

    ident = const_pool.tile([P, P], F32)
    make_identity(nc, ident[:])

    # ones_dbl[p, m] = 1 iff p == m (mod 64): one matmul against it both sums
    # the two column-tiled KV halves and replicates the result to partitions
    # 64..127 (needed as the row-group-1 operand of the row-tiled out matmuls).
    ones_dbl = const_pool.tile([P, P], F32)
    nc.gpsimd.memset(ones_dbl[:], 0.0)
    for off in (-64, 0, 64):
        nc.gpsimd.affine_select(
            out=ones_dbl[:],
            in_=ones_dbl[:],
            compare_op=mybir.AluOpType.not_equal,
            fill=1.0,
            base=-off,
            pattern=[[-1, P]],
            channel_multiplier=1,
        )

    for h in range(HPC):
        q_sb = in_pool.tile([P, J, D], F32, tag="q")
        k_sb = in_pool.tile([P, J, D], F32, tag="k")
        v_sb = in_pool.tile([P, J, D], F32, tag="v")
        # q first (transposes need only q + ident), halved so the first
        # transposes can start after 256 KiB instead of a full head.
        qv = q_d[h].rearrange("(p j) d -> p j d", p=P)
        nc.sync.dma_start(q_sb[:, 0 : J // 2], qv[:, 0 : J // 2])
        nc.sync.dma_start(q_sb[:, J // 2 : J], qv[:, J // 2 : J])
        nc.sync.dma_start(k_sb[:], k_d[h].rearrange("(p j) d -> p j d", p=P))
        nc.sync.dma_start(v_sb[:], v_d[h].rearrange("(p j) d -> p j d", p=P))

        # Transposes first in program order: they depend only on q, which
        # arrives first, so the PE has work before k/v land.
        qts = []
        for jp in range(J // 2):
            qt_ps = psum_t.tile([P, P], F32, tag="qt_ps")
            nc.tensor.transpose(qt_ps[:], q_sb[:, 2 * jp : 2 * jp + 2], ident[:])
            qt_sb = qt_pool.tile([P, P], F32, tag="qt")
            nc.any.tensor_copy(qt_sb[:], qt_ps[:])
            qts.append(qt_sb)

        # KV = k.T @ v, column-tiled: even j-slots accumulate into PE columns
        # 0..63 (psum partitions 0..63), odd slots into columns 64..127, so
        # the two matmuls of a pair run concurrently in the array.
        kv_ps = psum_kv.tile([P, D], F32)
        for jp in range(J // 2):
            nc.tensor.matmul(
                kv_ps[0:D],
                k_sb[:, 2 * jp],
                v_sb[:, 2 * jp],
                start=(jp == 0),
                stop=(jp == J // 2 - 1),
                tile_position=(0, 0),
                skip_group_check=True,
            )
            nc.tensor.matmul(
                kv_ps[D : 2 * D],
                k_sb[:, 2 * jp + 1],
                v_sb[:, 2 * jp + 1],
                start=(jp == 0),
                stop=(jp == J // 2 - 1),
                tile_position=(0, D),
                skip_group_check=True,
            )
        kv_raw = kv_pool.tile([P, D], F32, tag="kv_raw")
        nc.any.tensor_copy(kv_raw[:], kv_ps[:])
        kv_st_ps = psum_s.tile([P, D], F32)
        nc.tensor.matmul(kv_st_ps[:], ones_dbl[:], kv_raw[:], start=True, stop=True)
        kv_stack = kv_pool.tile([P, D], F32, tag="kv_stack")
        nc.any.tensor_copy(kv_stack[:], kv_st_ps[:])

        # out = q @ KV: one PE transpose yielded qT for slots 2jp/2jp+1 on
        # partition halves; two row-tiled matmuls (row groups 0..63 / 64..127)
        # run concurrently. Output store is split in halves to overlap the tail.
        out_sb = out_pool.tile([P, J, D], F32, tag="o")
        ov = o_d[h].rearrange("(p j) d -> p j d", p=P)
        for jp in range(J // 2):
            qt_sb = qts[jp]
            o_ps_a = psum_o.tile([P, D], F32, tag="o_ps_a")
            o_ps_b = psum_o.tile([P, D], F32, tag="o_ps_b")
            nc.tensor.matmul(
                o_ps_a[:],
                qt_sb[0:D],
                kv_stack[0:D],
                start=True,
                stop=True,
                tile_position=(0, 0),
            )
            nc.tensor.matmul(
                o_ps_b[:],
                qt_sb[D : 2 * D],
                kv_stack[D : 2 * D],
                start=True,
                stop=True,
                tile_position=(D, 0),
            )
            nc.any.tensor_copy(out_sb[:, 2 * jp], o_ps_a[:])
            nc.any.tensor_copy(out_sb[:, 2 * jp + 1], o_ps_b[:])
            if jp == J // 4 - 1:
                nc.sync.dma_start(ov[:, 0 : J // 2], out_sb[:, 0 : J // 2])
        nc.sync.dma_start(ov[:, J // 2 : J], out_sb[:, J // 2 : J])


def build():
    nc = bacc.Bacc("TRN2", target_bir_lowering=False, debug=False)
    q_d = nc.dram_tensor("q", [HPC, L, D], F32, kind="ExternalInput").ap()
    k_d = nc.dram_tensor("k", [HPC, L, D], F32, kind="ExternalInput").ap()
    v_d = nc.dram_tensor("v", [HPC, L, D], F32, kind="ExternalInput").ap()
    o_d = nc.dram_tensor("out", [HPC, L, D], F32, kind="ExternalOutput").ap()
    with tile.TileContext(nc) as tc, ExitStack() as ctx:
        _body(ctx, tc, o_d, q_d, k_d, v_d)
    nc.compile()
    return nc


_NC = None


def _get_nc():
    global _NC
    if _NC is None:
        _NC = build()
    return _NC


def make_in_maps(q, k, v):
    qf = np.ascontiguousarray(np.asarray(q, dtype=np.float32).reshape(B * H, L, D))
    kf = np.ascontiguousarray(np.asarray(k, dtype=np.float32).reshape(B * H, L, D))
    vf = np.ascontiguousarray(np.asarray(v, dtype=np.float32).reshape(B * H, L, D))
    return [
        {
            "q": np.ascontiguousarray(qf[c * HPC : (c + 1) * HPC]),
            "k": np.ascontiguousarray(kf[c * HPC : (c + 1) * HPC]),
            "v": np.ascontiguousarray(vf[c * HPC : (c + 1) * HPC]),
        }
        for c in range(N_CORES)
    ]


def run_sharded(q, k, v, **spmd_kwargs):
    """Run on all 8 cores; returns (full_output, BassKernelResults)."""
    nc = _get_nc()
    res = run_bass_kernel_spmd(
        nc, make_in_maps(q, k, v), core_ids=list(range(N_CORES)), **spmd_kwargs
    )
    shards = [np.asarray(res.results[c]["out"]) for c in range(N_CORES)]
    out = np.concatenate(shards, axis=0).reshape(B, H, L, D).astype(np.float32)
    return out, res


def kernel(q, k, v):
    out, _ = run_sharded(q, k, v)
    return out
